# revision 19
# baseline (speedup 1.0000x reference)
"""Trainium2 Bass kernel for a dense transformer decoder block.

Strategy (8 NeuronCores, tensor-parallel a la Megatron):
  - heads sharded across cores (H/8 heads each) for attention,
    FFN hidden dim sharded (HID/8 each).
  - Activations kept in transposed layout [D, tokens] on device so every
    matmul contracts over the partition dim with fp32r (full-rate fp32).
  - rmsnorm column sums via ones-matmul on the PE (replicated [128, t]
    stats, so no partition broadcasts are needed).
  - Residual x is folded into the wo AllReduce as x/8; the AllReduce
    output IS h.  The final residual h is folded into the w2
    ReduceScatter as h/8; the RS output IS the final result, sharded
    over D rows across cores.  Host reassembles + transposes.
  - Causality is not hardcoded: the mask input is classified host-side
    into skip / plain / mixed 128x512 blocks; mixed tiles are shipped
    as constants (4 distinct tiles for a causal mask).
"""

import os
import sys

try:  # the axon sitecustomize usually provides concourse already
    import concourse.bass  # noqa: F401
except ImportError:  # pragma: no cover
    sys.path.insert(0, "/opt/trn_rl_repo")

from contextlib import ExitStack

import numpy as np

import concourse.bacc as bacc
import concourse.tile as tile
from concourse import mybir
from concourse.bass_utils import run_bass_kernel_spmd

F32 = mybir.dt.float32
F32R = mybir.dt.float32r
N_CORES = 8
P = 128
QW = 512  # q-tile / token-tile width
EPS = 1e-6
AF = mybir.ActivationFunctionType


def ts(i, w):
    return slice(i * w, (i + 1) * w)


def _classify_mask(mask, S):
    """mask: [S, S] additive (q, k). Returns (table, tiles).
    table[(kt, j)] = 'skip' | 'plain' | int mask-tile index.
    tiles: list of [128, QW] float32 arrays in scoresT ([k, q]) layout."""
    table = {}
    tiles = []
    keys = {}
    for j in range(S // QW):
        for kt in range(S // P):
            sub = mask[ts(j, QW), ts(kt, P)]  # [q, k]
            if np.all(sub <= -1e8):
                table[(kt, j)] = "skip"
            elif np.all(sub == 0.0):
                table[(kt, j)] = "plain"
            else:
                t = np.ascontiguousarray(sub.T.astype(np.float32))  # [k, q]
                key = t.tobytes()
                if key not in keys:
                    keys[key] = len(tiles)
                    tiles.append(t)
                table[(kt, j)] = keys[key]
    return table, tiles


def build_program(B, S, D, H, HID, mask_table, n_mask):
    HD = 128
    assert D == (D // P) * P and H * HD == D
    HPC = H // N_CORES            # heads per core
    assert HPC * N_CORES == H
    C = D // P                    # contraction chunks over D
    S_TILES = S // QW             # q tiles per batch
    KT = S // P                   # k tiles per batch
    T = B * S                     # total tokens
    OC = HPC * HD // P            # wo input-channel chunks (== HPC)
    HIDC = HID // N_CORES // P    # hidden tiles per core
    HC = T // QW                  # half-chunk count (512-token tiles)
    N_CHUNKS = max(1, T // 1024)  # collective chunks
    CH_T = T // N_CHUNKS          # tokens per collective chunk
    DS = D // N_CORES             # output row shard per core

    nc = bacc.Bacc(trn_type="TRN2", num_devices=N_CORES)

    xt = nc.dram_tensor("xt", [B, D, S], F32, kind="ExternalInput").ap()
    wq = nc.dram_tensor("wq", [C, P, HPC * HD], F32, kind="ExternalInput").ap()
    wk = nc.dram_tensor("wk", [C, P, HPC * HD], F32, kind="ExternalInput").ap()
    wv = nc.dram_tensor("wv", [C, P, HPC * HD], F32, kind="ExternalInput").ap()
    wo = nc.dram_tensor("wo", [OC, P, D], F32, kind="ExternalInput").ap()
    w1 = nc.dram_tensor("w1", [C, P, HIDC * P], F32, kind="ExternalInput").ap()
    w2 = nc.dram_tensor("w2", [C, HIDC, P, P], F32, kind="ExternalInput").ap()
    mk = None
    if n_mask:
        mk = nc.dram_tensor("mk", [n_mask, P, QW], F32, kind="ExternalInput").ap()

    ar_in = [nc.dram_tensor(f"ar_in{k}", [D, CH_T], F32) for k in range(N_CHUNKS)]
    ar_out = [
        nc.dram_tensor(f"ar_out{k}", [D, CH_T], F32, addr_space="Shared")
        for k in range(N_CHUNKS)
    ]
    rs_in = [nc.dram_tensor(f"rs_in{k}", [D, CH_T], F32) for k in range(N_CHUNKS)]
    rs_out = [
        nc.dram_tensor(f"rs_out{k}", [DS, CH_T], F32) for k in range(N_CHUNKS)
    ]
    outs = [
        nc.dram_tensor(f"out{k}", [DS, CH_T], F32, kind="ExternalOutput")
        for k in range(N_CHUNKS)
    ]

    groups = [list(range(N_CORES))]

    def rb(ap):  # reinterpret an f32 DRAM source as f32r for DMA into f32r tiles
        return ap.bitcast(F32R)

    with tile.TileContext(nc) as tc, ExitStack() as ctx:
        const = ctx.enter_context(tc.tile_pool(name="const", bufs=1))
        stats = ctx.enter_context(tc.tile_pool(name="stats", bufs=2))
        sqp = ctx.enter_context(tc.tile_pool(name="sq", bufs=2))
        evp = ctx.enter_context(tc.tile_pool(name="ev", bufs=2))
        psum = ctx.enter_context(tc.tile_pool(name="psum", bufs=1, space="PSUM"))

        ones_f32 = const.tile([P, P], F32)
        nc.vector.memset(ones_f32[:], 1.0)
        ones = const.tile([P, P], F32R)
        nc.vector.tensor_copy(ones[:], ones_f32[:])
        eps_p1 = const.tile([P, 1], F32)
        nc.vector.memset(eps_p1[:], EPS)

        mtiles = None
        if n_mask:
            mtiles = const.tile([P, n_mask, QW], F32)
            nc.sync.dma_start(mtiles[:], mk.rearrange("n p q -> p n q"))

        # ---------------- attention weights (resident) ----------------
        AW = 256  # stage-A token-tile width
        with tc.tile_pool(name="wqkv", bufs=1) as wqkvp, \
             tc.tile_pool(name="xa", bufs=2) as xap, \
             tc.tile_pool(name="qkv", bufs=1) as qkvp, \
             tc.tile_pool(name="exp", bufs=3) as expp, \
             tc.tile_pool(name="attn", bufs=1) as attp, \
             tc.tile_pool(name="xres", bufs=2) as xrp:
            wq_sb = wqkvp.tile([P, C, HPC * HD], F32R, tag="wq")
            nc.sync.dma_start(wq_sb[:], rb(wq.rearrange("c p o -> p c o")))
            wk_sb = wqkvp.tile([P, C, HPC * HD], F32R, tag="wk")
            nc.sync.dma_start(wk_sb[:], rb(wk.rearrange("c p o -> p c o")))
            wv_sb = wqkvp.tile([P, C, HPC * HD], F32R, tag="wv")
            nc.sync.dma_start(wv_sb[:], rb(wv.rearrange("c p o -> p c o")))
            wo_sb = wqkvp.tile([P, OC, D], F32R, tag="wo")
            nc.sync.dma_start(wo_sb[:], rb(wo.rearrange("c p o -> p c o")))

            for b in range(B):
                # -------- stage A: rmsnorm1 + q/k/v projections --------
                qT = qkvp.tile([P, HPC, S], F32R, tag="qT")
                kT = qkvp.tile([P, HPC, S], F32R, tag="kT")
                vN = qkvp.tile([P, KT, HPC * HD], F32R, tag="vN")
                for st in range(S // AW):
                    xti = xap.tile([P, C, AW], F32R, tag="xa")
                    nc.sync.dma_start(
                        xti[:],
                        rb(xt[b].rearrange("(c p) t -> p c t", p=P)[:, :, ts(st, AW)]),
                    )
                    cs = psum.tile([P, QW], F32, tag="stat", bufs=1)
                    for c in range(C):
                        sq = sqp.tile([P, AW], F32R, tag="sq")
                        nc.scalar.activation(sq[:], xti[:, c, :], AF.Square)
                        nc.tensor.matmul(
                            cs[:, :AW], ones[:], sq[:], start=(c == 0), stop=(c == C - 1)
                        )
                    rms = stats.tile([P, QW], F32, tag="rms")
                    nc.scalar.activation(
                        rms[:, :AW], cs[:, :AW], AF.Sqrt, bias=eps_p1[:], scale=1.0 / D
                    )
                    rinv = stats.tile([P, QW], F32, tag="rinv")
                    nc.vector.reciprocal(rinv[:, :AW], rms[:, :AW])
                    for c in range(C):
                        nc.vector.tensor_mul(xti[:, c, :], xti[:, c, :], rinv[:, :AW])
                    # q/k projections, transposed layout [HD, tokens]
                    for h in range(HPC):
                        for w_sb, dst in ((wq_sb, qT), (wk_sb, kT)):
                            pp = psum.tile([P, QW], F32, tag="mm", bufs=3)
                            for c in range(C):
                                nc.tensor.matmul(
                                    pp[:, :AW],
                                    w_sb[:, c, ts(h, HD)],
                                    xti[:, c, :],
                                    start=(c == 0),
                                    stop=(c == C - 1),
                                )
                            nc.vector.tensor_copy(dst[:, h, ts(st, AW)], pp[:, :AW])
                    # v in natural layout [tokens, HPC*HD]
                    for sub in range(AW // P):
                        pv = psum.tile([P, QW], F32, tag="mm", bufs=3)
                        for c in range(C):
                            nc.tensor.matmul(
                                pv[:, : HPC * HD],
                                xti[:, c, ts(sub, P)],
                                wv_sb[:, c, :],
                                start=(c == 0),
                                stop=(c == C - 1),
                            )
                        nc.vector.tensor_copy(
                            vN[:, st * (AW // P) + sub, :], pv[:, : HPC * HD]
                        )

                # -------- stage B+C: attention, wo partial fused per q-tile --------
                for j in range(S_TILES):
                    attnT = attp.tile([P, HPC, QW], F32R, tag="attnT", bufs=2)
                    for h in range(HPC):
                        kts = [
                            kt for kt in range(KT) if mask_table[(kt, j)] != "skip"
                        ]
                        pa = psum.tile([P, QW], F32, tag="pv", bufs=2)
                        den = psum.tile([P, QW], F32, tag="stat", bufs=1)
                        for i, kt in enumerate(kts):
                            msc = psum.tile([P, QW], F32, tag="score", bufs=2)
                            nc.tensor.matmul(
                                msc[:],
                                kT[:, h, ts(kt, P)],
                                qT[:, h, ts(j, QW)],
                                start=True,
                                stop=True,
                            )
                            ex = expp.tile([P, QW], F32R, tag="exp")
                            mt = mask_table[(kt, j)]
                            if mt == "plain":
                                nc.scalar.activation(ex[:], msc[:], AF.Exp)
                            else:
                                nc.vector.tensor_add(ex[:], msc[:], mtiles[:, mt, :])
                                nc.scalar.activation(ex[:], ex[:], AF.Exp)
                            nc.tensor.matmul(
                                den[:], ones[:], ex[:],
                                start=(i == 0), stop=(i == len(kts) - 1),
                            )
                            nc.tensor.matmul(
                                pa[:],
                                vN[:, kt, ts(h, HD)],
                                ex[:],
                                start=(i == 0),
                                stop=(i == len(kts) - 1),
                            )
                        rec = stats.tile([P, QW], F32, tag="rms")
                        nc.vector.reciprocal(rec[:], den[:])
                        nc.vector.tensor_mul(attnT[:, h, :], pa[:], rec[:])

                    # wo partial + x/8 -> AR input for this q-tile
                    g = b * S + j * QW  # global token offset
                    k = g // CH_T
                    off = g % CH_T
                    for ot in range(C):
                        po = psum.tile([P, QW], F32, tag="mm", bufs=3)
                        for oc in range(OC):
                            nc.tensor.matmul(
                                po[:],
                                wo_sb[:, oc, ts(ot, P)],
                                attnT[:, oc, :],
                                start=(oc == 0),
                                stop=(oc == OC - 1),
                            )
                        xres = xrp.tile([P, QW], F32, tag="xres")
                        nc.scalar.dma_start(xres[:], xt[b, ts(ot, P), ts(j, QW)])
                        ev = evp.tile([P, QW], F32, tag="ev")
                        nc.vector.scalar_tensor_tensor(
                            ev[:], xres[:], 1.0 / N_CORES, po[:],
                            op0=mybir.AluOpType.mult, op1=mybir.AluOpType.add,
                        )
                        nc.scalar.dma_start(
                            ar_in[k].ap()[ts(ot, P), off : off + QW], ev[:]
                        )

        for k in range(N_CHUNKS):
            nc.gpsimd.collective_compute(
                "AllReduce",
                mybir.AluOpType.add,
                replica_groups=groups,
                ins=[ar_in[k].ap().opt()],
                outs=[ar_out[k].ap().opt()],
            )

        # ---------------- FFN phase (h = ar_out) ----------------
        with tc.tile_pool(name="wffn", bufs=1) as wffnp, \
             tc.tile_pool(name="w2s", bufs=3) as w2sp, \
             tc.tile_pool(name="hf", bufs=2) as hfp, \
             tc.tile_pool(name="up", bufs=2) as upp:
            w1_sb = wffnp.tile([P, C, HIDC * P], F32R, tag="w1")
            nc.sync.dma_start(w1_sb[:], rb(w1.rearrange("c p o -> p c o")))

            for hc in range(HC):
                k = hc * QW // CH_T
                off = (hc * QW) % CH_T
                ht = hfp.tile([P, C, QW], F32R, tag="hf")
                nc.sync.dma_start(
                    ht[:],
                    rb(
                        ar_out[k].ap().rearrange("(c p) t -> p c t", p=P)[
                            :, :, off : off + QW
                        ]
                    ),
                )
                # rmsnorm2 stats
                cs = psum.tile([P, QW], F32, tag="stat", bufs=1)
                for c in range(C):
                    sq = sqp.tile([P, QW], F32R, tag="sq")
                    nc.scalar.activation(sq[:], ht[:, c, :], AF.Square)
                    nc.tensor.matmul(
                        cs[:], ones[:], sq[:], start=(c == 0), stop=(c == C - 1)
                    )
                rms = stats.tile([P, QW], F32, tag="rms")
                nc.scalar.activation(
                    rms[:], cs[:], AF.Sqrt, bias=eps_p1[:], scale=1.0 / D
                )
                r2 = stats.tile([P, QW], F32, tag="rinv")
                nc.vector.reciprocal(r2[:], rms[:])

                # up = relu(w1^T h) * r2  (r2 applied post-relu; valid as r2>0)
                up = upp.tile([P, HIDC, QW], F32R, tag="up")
                for ht_i in range(HIDC):
                    pu = psum.tile([P, QW], F32, tag="mm", bufs=3)
                    for c in range(C):
                        nc.tensor.matmul(
                            pu[:],
                            w1_sb[:, c, ts(ht_i, P)],
                            ht[:, c, :],
                            start=(c == 0),
                            stop=(c == C - 1),
                        )
                    nc.scalar.activation(up[:, ht_i, :], pu[:], AF.Relu)
                    nc.vector.tensor_mul(up[:, ht_i, :], up[:, ht_i, :], r2[:])

                # down partial + h/8 -> RS input
                for ot in range(C):
                    w2t = w2sp.tile([P, HIDC, P], F32R, tag="w2t")
                    nc.sync.dma_start(w2t[:], rb(w2[ot].rearrange("c p o -> p c o")))
                    pd = psum.tile([P, QW], F32, tag="mm", bufs=3)
                    for c in range(HIDC):
                        nc.tensor.matmul(
                            pd[:],
                            w2t[:, c, :],
                            up[:, c, :],
                            start=(c == 0),
                            stop=(c == HIDC - 1),
                        )
                    ev = evp.tile([P, QW], F32, tag="ev")
                    nc.vector.scalar_tensor_tensor(
                        ev[:], ht[:, ot, :], 1.0 / N_CORES, pd[:],
                        op0=mybir.AluOpType.mult, op1=mybir.AluOpType.add,
                    )
                    nc.scalar.dma_start(rs_in[k].ap()[ts(ot, P), off : off + QW], ev[:])

        for k in range(N_CHUNKS):
            nc.gpsimd.collective_compute(
                "ReduceScatter",
                mybir.AluOpType.add,
                replica_groups=groups,
                ins=[rs_in[k].ap().opt()],
                outs=[rs_out[k].ap().opt()],
            )
            nc.sync.dma_start(outs[k].ap(), rs_out[k].ap())

    nc.compile()
    return nc, N_CHUNKS, CH_T, DS


_CACHE = {}
LAST_RESULT = None


def _get_program(B, S, D, H, HID, mask_table, n_mask, mask_key):
    key = (B, S, D, H, HID, mask_key)
    if key not in _CACHE:
        _CACHE[key] = build_program(B, S, D, H, HID, mask_table, n_mask)
    return _CACHE[key]


def kernel(x, mask, wq, wk, wv, wo, w1, w2, attn_norm_w, ffn_norm_w):
    x = np.asarray(x, dtype=np.float32)
    mask = np.asarray(mask, dtype=np.float32)
    wq, wk, wv, wo = (np.asarray(a, dtype=np.float32) for a in (wq, wk, wv, wo))
    w1, w2 = np.asarray(w1, dtype=np.float32), np.asarray(w2, dtype=np.float32)
    attn_norm_w = np.asarray(attn_norm_w, dtype=np.float32)
    ffn_norm_w = np.asarray(ffn_norm_w, dtype=np.float32)

    B, S, D = x.shape
    H = D // 128  # HD is fixed at 128 (= SBUF partition count)
    HID = w1.shape[0]
    HD = D // H
    HPC = H // N_CORES
    C = D // P
    HIDC = HID // N_CORES // P

    mask_table, mtiles_np = _classify_mask(
        np.broadcast_to(mask, (1, 1, S, S))[0, 0], S
    )
    mask_key = hash(tuple(sorted((k, str(v)) for k, v in mask_table.items())))
    nc, N_CHUNKS, CH_T, DS = _get_program(
        B, S, D, H, HID, mask_table, len(mtiles_np), mask_key
    )

    # ---- host-side prep ----
    xt = np.ascontiguousarray(x.transpose(0, 2, 1))  # [B, D, S]
    wq_f = (wq * attn_norm_w[None, :]) / np.sqrt(HD)
    wk_f = wk * attn_norm_w[None, :]
    wv_f = wv
    w1_f = w1 * ffn_norm_w[None, :]

    in_maps = []
    for c in range(N_CORES):
        hs = slice(c * HPC * HD, (c + 1) * HPC * HD)
        qs = np.ascontiguousarray(wq_f[hs].T).reshape(C, P, HPC * HD)
        ks = np.ascontiguousarray(wk_f[hs].T).reshape(C, P, HPC * HD)
        vs = np.ascontiguousarray(wv_f[hs].T).reshape(C, P, HPC * HD)
        os_ = np.ascontiguousarray(wo[:, hs].T).reshape(HPC, P, D)
        fs = slice(c * HIDC * P, (c + 1) * HIDC * P)
        w1s = np.ascontiguousarray(w1_f[fs].T).reshape(C, P, HIDC * P)
        # w2 shard -> [ot, hid_c, p, o] pre-chunked
        w2t = np.ascontiguousarray(w2[:, fs].T)  # [HIDC*P, D]
        w2r = np.ascontiguousarray(
            w2t.reshape(HIDC, P, C, P).transpose(2, 0, 1, 3)
        )  # [C, HIDC, P, P]
        m = {
            "xt": xt,
            "wq": qs,
            "wk": ks,
            "wv": vs,
            "wo": os_,
            "w1": w1s,
            "w2": w2r,
        }
        if len(mtiles_np):
            m["mk"] = np.stack(mtiles_np)
        in_maps.append(m)

    trace = os.environ.get("KTRACE", "0") == "1"
    res = run_bass_kernel_spmd(nc, in_maps, list(range(N_CORES)), trace=trace)
    global LAST_RESULT
    LAST_RESULT = res

    out_T = np.empty((D, B * S), dtype=np.float32)
    for r_ in range(N_CORES):
        for k in range(N_CHUNKS):
            out_T[r_ * DS : (r_ + 1) * DS, k * CH_T : (k + 1) * CH_T] = res.results[
                r_
            ][f"out{k}"]
    return np.ascontiguousarray(out_T.reshape(D, B, S).transpose(1, 2, 0))
